# revision 18
# baseline (speedup 1.0000x reference)
"""Trainium2 Bass kernel for nn_BinaryDiceLoss (sum of per-pixel BCE).

loss = sum_{b,h,w} mean_c[-(t*log(p) + (1-t)*log(1-p))], shapes [32,1,1024,1024] f32.

Sharding: data-parallel over batch - 4 images (4.19M elements) per NeuronCore
on 8 cores.

Host canonicalizes each element to p' = max(p, 1-p), t' = (p>=0.5 ? t : 1-t)
(bce is symmetric under (p,t)->(1-p,1-t)), then quantizes p' to u8:
c = floor(256*p') in [128,255], p_hat = (c+0.5)/256 in [0.5,1).  p_hat spans
ONE fp16 binade, so bits_i16(fp16(p_hat)) = 13316+8c exactly and the fp16
log-bit-hack ln(x) ~= A*bits+B makes  v := -log(p') = K - 8A*c  affine in c.
Identity: bce = t'*(log1mp' - logp') - log1mp', u = lg + v.

Three per-segment pipelines, balancing ACT (1x, the old bottleneck), DVE and
the PE array; column 127 of every ones-row chunk's stationary is baked to 1.0
so PSUM ROW 127 accumulates sum(lg) for free (no ACT accum needed there);
the sacrificed elements are reprocessed in a tiny duplicate segment D1:
  cfg i   (edges):   ACT lg->uv blk0 | DVE v->uv blk1 | MM FD=256 -> psum1
  cfg ii:            ACT lg(+accum)  | DVE v, lg+=v   | MM FD=128 -> psum2
  cfg iii:           DVE y=(255.5-c)/256 fp16, hk=A*bits(y)+B->uv blk0 (NO
                     ACT at all), v->uv blk1          | MM FD=256 -> psum1
  D1 (dup cols):     like ii, no accum (their lg is already counted by the
                     ones-row), real t' stationary -> psum2
Host: total = diag(psum1 excl row127) + diag(psum2) - psum1[127,:128].sum()
              - sum(cfg-ii accums).   Expected rel err ~1.2-1.5e-3 (u8
quantization bias; all other errors mean-zero), vs the 2e-2 gate.
1B predict + 1B target + 0.8% dup = 2B/elem DMA.
"""

import math
import os

import numpy as np

_N_CORES = 8
_P = 128
_PER_CORE = 32 * 1024 * 1024 // _N_CORES // _P  # 32768 columns of 128
_SEGS = [1024, 1024, 2048] + [4096] * 6 + [2048, 1024, 1024]
assert sum(_SEGS) == _PER_CORE

_CFG_II = set(
    int(x) for x in os.environ.get("K_CFG_II", "3,4,5,6").split(",") if x
)
_CFG_III = set(
    int(x) for x in os.environ.get("K_CFG_III", "7,8").split(",") if x
)
assert not (_CFG_II & _CFG_III)
_IO_BUFS = int(os.environ.get("K_IO_BUFS", "4"))
_WK_BUFS = int(os.environ.get("K_WK_BUFS", "3"))

_LN2 = math.log(2.0)
_A = _LN2 / 1024.0
_B = -15.0 * _LN2 + (1.5 * _LN2 - 1.0)
_K = -(13316.0 * _A + _B)  # v = K - 8A*c = -log(p') under the bit-hack

# chunks whose stationary col-127 is sacrificed (ones-row segments)
_ONES_SEGS = [s for s in range(len(_SEGS)) if s not in _CFG_II]
_N_D1 = sum(_SEGS[s] // _P for s in _ONES_SEGS)  # sacrificed columns
_D1_PAD = (-_N_D1) % _P
_D1_FL = _N_D1 + _D1_PAD

_CACHED_NC = None
LAST_RESULTS = None  # BassKernelResults of the most recent run (for harnesses)


def _seg_classes():
    counts = {}
    for fl in _SEGS:
        counts[fl] = counts.get(fl, 0) + 1
    return counts


def _build():
    import concourse.bacc as bacc
    import concourse.tile as tile
    from concourse import mybir

    f32 = mybir.dt.float32
    bf16 = mybir.dt.bfloat16
    fp16 = mybir.dt.float16
    i16 = mybir.dt.int16
    u8 = mybir.dt.uint8
    fp8 = mybir.dt.float8e4
    Alu = mybir.AluOpType
    p = _P

    nc = bacc.Bacc(
        "TRN2",
        target_bir_lowering=False,
        debug=False,
        enable_asserts=False,
        num_devices=_N_CORES,
    )
    counts = _seg_classes()
    pred = {
        fl: nc.dram_tensor(f"p{fl}", [n, p, fl], u8, kind="ExternalInput").ap()
        for fl, n in counts.items()
    }
    targ = {
        fl: nc.dram_tensor(f"t{fl}", [n, p, fl], fp8, kind="ExternalInput").ap()
        for fl, n in counts.items()
    }
    pd1 = nc.dram_tensor("pd1", [p, _D1_FL], u8, kind="ExternalInput").ap()
    td1 = nc.dram_tensor("td1", [p, _D1_FL], fp8, kind="ExternalInput").ap()
    nseg = len(_SEGS)
    out_b = nc.dram_tensor("out_b", [p, nseg], f32, kind="ExternalOutput").ap()
    out_d1 = nc.dram_tensor("out_d1", [p, 2 * p], f32, kind="ExternalOutput").ap()
    out_d2 = nc.dram_tensor("out_d2", [p, p], f32, kind="ExternalOutput").ap()

    io_bufs = {1024: 4, 2048: 3, 4096: _IO_BUFS}
    wk_bufs = {1024: 2, 2048: 2, 4096: _WK_BUFS}

    with tile.TileContext(nc) as tc:
        with (
            tc.tile_pool(name="cin", bufs=1) as cin,
            tc.tile_pool(name="tin", bufs=1) as tin,
            tc.tile_pool(name="uv", bufs=1) as uvp,
            tc.tile_pool(name="lgc", bufs=1) as lgcp,
            tc.tile_pool(name="vc", bufs=1) as vcp,
            tc.tile_pool(name="yy", bufs=1) as yyp,
            tc.tile_pool(name="accs", bufs=1) as accs,
            tc.tile_pool(name="ps", bufs=1, space="PSUM") as ps,
        ):
            asums = accs.tile([p, nseg], f32, tag="asums")
            qb = accs.tile([p, 1], f32, tag="qb")
            nc.gpsimd.memset(qb, 255.5 / 256.0)
            warm = accs.tile([p, 1], fp16, tag="warm")
            nc.scalar.activation(
                out=warm, in_=qb, func=mybir.ActivationFunctionType.Ln,
                bias=qb[:, :], scale=0.0,
            )
            psum1 = ps.tile([p, 2 * p], f32, tag="psum1")
            psum2 = ps.tile([p, p], f32, tag="psum2")

            cls_idx = {fl: 0 for fl in counts}
            cts = {}

            def fetch_c(s):
                fl = _SEGS[s]
                i = cls_idx[fl]
                ct = cin.tile([p, fl], u8, tag=f"c{fl}", bufs=io_bufs[fl])
                nc.sync.dma_start(out=ct, in_=pred[fl][i, :, :])
                cts[s] = (ct, fl, i)
                cls_idx[fl] = i + 1

            ones_mm = [(s, c) for s in _ONES_SEGS
                       for c in range(_SEGS[s] // p)]
            ii_first = min(_CFG_II) if _CFG_II else None

            fetch_c(0)
            fetch_c(1)
            for s in range(nseg):
                if s + 2 < nseg:
                    fetch_c(s + 2)
                ct, fl, i = cts.pop(s)
                wt = tin.tile([p, fl], fp8, tag=f"t{fl}", bufs=io_bufs[fl])
                nc.sync.dma_start(out=wt, in_=targ[fl][i, :, :])
                nch = fl // p
                if s in _CFG_II:
                    # contiguous lg/v; add on DVE; FD=128 MMs into psum2
                    lg = lgcp.tile([p, fl], bf16, tag=f"lg{fl}",
                                   bufs=wk_bufs[fl])
                    nc.scalar.activation(
                        out=lg, in_=ct, func=mybir.ActivationFunctionType.Ln,
                        bias=qb[:, :], scale=-1.0 / 256.0,
                        accum_out=asums[:, s:s + 1],
                    )
                    v = vcp.tile([p, fl], bf16, tag=f"v{fl}",
                                 bufs=wk_bufs[fl])
                    nc.vector.tensor_scalar(v, ct, -8.0 * _A, _K,
                                            Alu.mult, Alu.add)
                    nc.vector.tensor_add(lg, lg, v)  # u, in place
                    for c in range(nch):
                        sl = slice(c * p, (c + 1) * p)
                        nc.tensor.matmul(
                            psum2[:, :], wt[:, sl], lg[:, sl],
                            start=(s == ii_first and c == 0), stop=False,
                        )
                else:
                    uv = uvp.tile([p, nch, 2, p], bf16, tag=f"uv{fl}",
                                  bufs=wk_bufs[fl])
                    if s in _CFG_III:
                        # no ACT: log1mp' via the fp16 bit-hack
                        y = yyp.tile([p, fl], fp16, tag=f"y{fl}",
                                     bufs=wk_bufs[fl])
                        nc.vector.tensor_scalar(y, ct, -1.0 / 256.0,
                                                255.5 / 256.0,
                                                Alu.mult, Alu.add)
                        nc.vector.tensor_scalar(uv[:, :, 0, :],
                                                y.bitcast(i16), _A, _B,
                                                Alu.mult, Alu.add)
                    else:
                        nc.scalar.activation(
                            out=uv[:, :, 0, :], in_=ct,
                            func=mybir.ActivationFunctionType.Ln,
                            bias=qb[:, :], scale=-1.0 / 256.0,
                        )
                    nc.vector.tensor_scalar(uv[:, :, 1, :], ct, -8.0 * _A,
                                            _K, Alu.mult, Alu.add)
                    first = (s, 0) == ones_mm[0]
                    for c in range(nch):
                        nc.tensor.matmul(
                            psum1[:, :], wt[:, c * p:(c + 1) * p],
                            uv[:, c, :, :],
                            start=(first and c == 0),
                            stop=((s, c) == ones_mm[-1]),
                        )
            # D1: sacrificed col-127 elements, real t', like cfg ii
            cd = cin.tile([p, _D1_FL], u8, tag="cd1")
            nc.sync.dma_start(out=cd, in_=pd1)
            wd = tin.tile([p, _D1_FL], fp8, tag="td1")
            nc.sync.dma_start(out=wd, in_=td1)
            lgd = lgcp.tile([p, _D1_FL], bf16, tag="lgd1")
            nc.scalar.activation(
                out=lgd, in_=cd, func=mybir.ActivationFunctionType.Ln,
                bias=qb[:, :], scale=-1.0 / 256.0,
            )
            vd = vcp.tile([p, _D1_FL], bf16, tag="vd1")
            nc.vector.tensor_scalar(vd, cd, -8.0 * _A, _K, Alu.mult, Alu.add)
            nc.vector.tensor_add(lgd, lgd, vd)
            for c in range(_D1_FL // p):
                sl = slice(c * p, (c + 1) * p)
                nc.tensor.matmul(
                    psum2[:, :], wd[:, sl], lgd[:, sl],
                    start=(ii_first is None and c == 0),
                    stop=(c == _D1_FL // p - 1),
                )
            nc.sync.dma_start(out=out_b, in_=asums, single_packet=True)
            d1c = accs.tile([p, 2 * p], f32, tag="d1c")
            nc.vector.tensor_copy(d1c, psum1)
            nc.sync.dma_start(out=out_d1, in_=d1c, single_packet=True)
            d2c = accs.tile([p, p], f32, tag="d2c")
            nc.vector.tensor_copy(d2c, psum2)
            nc.sync.dma_start(out=out_d2, in_=d2c, single_packet=True)

    nc.compile()
    return nc


def kernel(predict: np.ndarray, target: np.ndarray, _trace: bool = False) -> np.ndarray:
    global _CACHED_NC, LAST_RESULTS
    from concourse.bass_utils import run_bass_kernel_spmd
    import ml_dtypes

    predict = np.asarray(predict)
    target = np.asarray(target)
    assert predict.shape == (32, 1, 1024, 1024) and predict.dtype == np.float32
    assert target.shape == (32, 1, 1024, 1024) and target.dtype == np.float32

    if _CACHED_NC is None:
        _CACHED_NC = _build()
    nc = _CACHED_NC

    counts = _seg_classes()
    pr = np.ascontiguousarray(predict).reshape(_N_CORES, _PER_CORE * _P)
    tg = np.ascontiguousarray(target).reshape(_N_CORES, _PER_CORE * _P)
    c0 = (pr * 256.0).astype(np.uint8)
    flip = c0 < 128
    cc = np.where(flip, 255 - c0, c0)                      # c' in [128,255]
    tt = np.where(flip, 1.0 - tg, tg).astype(np.float32)   # t'
    t8 = tt.astype(ml_dtypes.float8_e4m3)
    one8 = ml_dtypes.float8_e4m3(1.0)

    in_maps = [dict() for _ in range(_N_CORES)]
    off = 0
    cls_i = {fl: 0 for fl in counts}
    segs_np = {
        fl: (np.empty((_N_CORES, n, _P, fl), np.uint8),
             np.empty((_N_CORES, n, _P, fl), ml_dtypes.float8_e4m3))
        for fl, n in counts.items()
    }
    cd1 = np.zeros((_N_CORES, _P, _D1_FL), np.uint8) + 128
    td1 = np.zeros((_N_CORES, _P, _D1_FL), ml_dtypes.float8_e4m3)
    d1_at = 0
    for s, fl in enumerate(_SEGS):
        n = _P * fl
        i = cls_i[fl]
        cseg = cc[:, off:off + n].reshape(_N_CORES, _P, fl)
        tseg = t8[:, off:off + n].reshape(_N_CORES, _P, fl).copy()
        if s not in _CFG_II:
            # sacrifice col 127 of each chunk: dup into D1, bake ones
            nch = fl // _P
            cols = [c * _P + 127 for c in range(nch)]
            cd1[:, :, d1_at:d1_at + nch] = cseg[:, :, cols]
            td1[:, :, d1_at:d1_at + nch] = tseg[:, :, cols]
            tseg[:, :, cols] = one8
            d1_at += nch
        segs_np[fl][0][:, i] = cseg
        segs_np[fl][1][:, i] = tseg
        cls_i[fl] = i + 1
        off += n
    assert d1_at == _N_D1
    for c in range(_N_CORES):
        for fl in counts:
            in_maps[c][f"p{fl}"] = segs_np[fl][0][c]
            in_maps[c][f"t{fl}"] = segs_np[fl][1][c]
        in_maps[c]["pd1"] = cd1[c]
        in_maps[c]["td1"] = td1[c]

    res = run_bass_kernel_spmd(
        nc, in_maps, core_ids=list(range(_N_CORES)), trace=_trace,
    )
    LAST_RESULTS = res
    total = 0.0
    for c in range(_N_CORES):
        d1 = np.asarray(res.results[c]["out_d1"], dtype=np.float64)
        d2 = np.asarray(res.results[c]["out_d2"], dtype=np.float64)
        b = np.asarray(res.results[c]["out_b"], dtype=np.float64)
        total += float(np.trace(d1[:, :_P])) - d1[127, 127]
        total += float(np.trace(d1[:, _P:])) - d1[127, _P + 127]
        total += float(np.trace(d2))
        total -= float(d1[127, 0:_P].sum())
        total -= float(sum(b[:, s].sum() for s in _CFG_II))
    return np.array(total, dtype=np.float32)
